# revision 1
# baseline (speedup 1.0000x reference)
"""Trainium2 Bass kernel for CodeAttention (B=4, S=2048, E=768, H=12).

Sharding: 8 cores = 4 batches x 2 head-groups (6 heads each).
Each core computes a partial projection output for its batch; the host
sums the two partials per batch and adds the (host-folded) bias row.
"""

import sys

if "/opt/trn_rl_repo" not in sys.path:
    sys.path.insert(0, "/opt/trn_rl_repo")

import numpy as np

import concourse.bass as bass  # noqa: F401  (engine types referenced via nc)
import concourse.mybir as mybir
import concourse.tile as tile
from concourse import bacc
from concourse.alu_op_type import AluOpType
from concourse.bass_utils import run_bass_kernel_spmd
from concourse.masks import make_identity

F32 = mybir.dt.float32
F32R = mybir.dt.float32r
Act = mybir.ActivationFunctionType

B, S, E, H, D = 4, 2048, 768, 12, 64
HC = 6                    # heads per core
QKC = HC * D * 2          # qk columns per core = 768
VC = HC * D               # v columns per core = 384
KCH = E // 128            # contraction chunks over E = 6
NKC = S // 128            # key chunks = 16
NQB = S // 512            # q blocks of 512 = 4
NSB = S // 512            # s blocks of 512 = 4
VW = D + 1                # v width incl. ones column = 65
MASK_NEG = -50.0


def build_program():
    nc = bacc.Bacc("TRN2", target_bir_lowering=False, debug=False, num_devices=8)

    x_d = nc.dram_tensor("x", [S, E], F32, kind="ExternalInput")
    wqk_d = nc.dram_tensor("wqk", [KCH, 128, QKC], F32R, kind="ExternalInput")
    wv_d = nc.dram_tensor("wv", [KCH, 128, VC], F32R, kind="ExternalInput")
    wp_d = nc.dram_tensor("wp", [VC // 128, 128, E], F32R, kind="ExternalInput")
    bqk_d = nc.dram_tensor("bqk", [QKC], F32, kind="ExternalInput")
    mb_d = nc.dram_tensor("mb", [S], F32, kind="ExternalInput")
    y_d = nc.dram_tensor("y", [S, E], F32, kind="ExternalOutput")

    with tile.TileContext(nc) as tc:
        _emit(nc, tc, x_d, wqk_d, wv_d, wp_d, bqk_d, mb_d, y_d)
    nc.compile()
    return nc


def _emit(nc, tc, x_d, wqk_d, wv_d, wp_d, bqk_d, mb_d, y_d):
    ctx_pools = []

    def pool(name, bufs, space="SBUF"):
        p = tc.tile_pool(name=name, bufs=bufs, space=space)
        ctx_pools.append(p)
        return p.__enter__()

    consts = pool("consts", 1)
    store = pool("store", 1)

    ident = consts.tile([128, 128], F32)
    make_identity(nc, ident[:])
    ones_row_f = consts.tile([1, D], F32)
    nc.vector.memset(ones_row_f[:], 1.0)
    ones_row = consts.tile([1, D], F32R)
    nc.vector.tensor_copy(ones_row[:], ones_row_f[:])

    # weights go over the SWDGE (gpsimd) queue so the x-chunk loads on the
    # sync HWDGE queue aren't serialized behind 4.7MB of weight traffic.
    wqk = consts.tile([128, KCH, QKC], F32R)
    wv = consts.tile([128, KCH, VC], F32R)
    wp = consts.tile([128, VC // 128, E], F32R)
    for k in range(KCH):
        nc.gpsimd.dma_start(wv[:, k, :], wv_d.ap()[k])
    for k in range(KCH):
        nc.gpsimd.dma_start(wqk[:, k, :], wqk_d.ap()[k])
    for t in range(VC // 128):
        nc.gpsimd.dma_start(wp[:, t, :], wp_d.ap()[t])

    bqk = consts.tile([128, QKC // 128], F32)
    nc.scalar.dma_start(bqk[:], bqk_d.ap().rearrange("(c p) -> p c", p=128))
    mb = consts.tile([128, NKC], F32)
    nc.scalar.dma_start(mb[:], mb_d.ap().rearrange("(c p) -> p c", p=128))

    # qkT store, one tile per s-block so attention deps are per-block:
    # tile m of 6 holds W-columns m*128..; q cols 0..383, k cols 384..767.
    qkT = [
        store.tile([128, QKC // 128, 512], F32R, name=f"qkT{sb}")
        for sb in range(NSB)
    ]
    # v store: per s-block [s-chunk, head, 65] with ones in column 64.
    vst = [
        store.tile([128, 4, HC, VW], F32R, name=f"vst{sb}") for sb in range(NSB)
    ]
    ones_f = consts.tile([128, 4 * HC], F32)
    nc.vector.memset(ones_f[:], 1.0)
    for sb in range(NSB):
        nc.vector.tensor_copy(
            vst[sb][:, :, :, D : D + 1],
            ones_f[:].rearrange("p (a b one) -> p a b one", a=4, b=HC, one=1),
        )
    # attn output (transposed): tile t rows = head dims 2t,2t+1.
    att = store.tile([128, VC // 128, S], F32R)

    # ---- Phase 1: QKV projections ----
    with (
        tc.tile_pool(name="xs", bufs=3) as xs_p,
        tc.tile_pool(name="xt", bufs=3) as xt_p,
        tc.tile_pool(name="tp", bufs=3, space="PSUM") as tp_p,
        tc.tile_pool(name="va", bufs=2, space="PSUM") as va_p,
        tc.tile_pool(name="qk", bufs=3, space="PSUM") as qk_p,
    ):
        _emit_qkv(nc, x_d, ident, wqk, wv, bqk, qkT, vst, xs_p, xt_p, tp_p, va_p, qk_p)

    # ---- Phase 2: attention + projection ----
    st_p = pool("st", 2, space="PSUM")       # [128,1024] = 2 banks each
    pv_p = pool("pv", 3, space="PSUM")
    misc_p = pool("miscp", 1, space="PSUM")  # shared bc/ya slot
    pt_p = pool("pt", 3)
    se_p = pool("se", 2)
    rb_p = pool("rb", 2)
    ys_p = pool("ys", 2)

    for qb in range(NQB):
        qs = slice(qb * 512, (qb + 1) * 512)
        deferred_norm = None
        for hp in range(HC // 2):
            pvs = [
                pv_p.tile([128, 512], F32, tag="pv", name=f"pv{qb}_{hp}_{i}")
                for i in range(2)
            ]
            for kc in range(NKC):
                # both heads of the pair share one 2-bank score tile so a
                # single (cheaper) exp covers them: free dim 1024 amortizes
                # ACT's per-instruction overhead.
                st = st_p.tile([128, 1024], F32, tag="st")
                for sub in range(2):
                    r0 = sub * 64
                    kb, ko = kc // 4, kc % 4
                    nc.tensor.matmul(
                        st[:, sub * 512 : (sub + 1) * 512],
                        qkT[kb][r0 : r0 + 64, 3 + hp, ko * 128 : (ko + 1) * 128],
                        qkT[qb][r0 : r0 + 64, hp, :],
                        start=True, stop=True,
                    )
                pt = pt_p.tile([128, 1024], F32R, tag="pt")
                nc.scalar.activation(
                    pt[:], st[:], Act.Exp, bias=mb[:, kc : kc + 1], scale=0.125
                )
                for sub in range(2):
                    h = hp * 2 + sub
                    nc.tensor.matmul(
                        pvs[sub][0:VW, :], vst[kc // 4][:, kc % 4, h, :],
                        pt[:, sub * 512 : (sub + 1) * 512],
                        start=(kc == 0), stop=(kc == NKC - 1),
                    )
                if kc == 1 and deferred_norm is not None:
                    deferred_norm()
                    deferred_norm = None
            def norm_pair(pvs=pvs, hp=hp, qs=qs):
                for sub in range(2):
                    se = se_p.tile([1, 512], F32R, tag="se", name="se")
                    nc.vector.tensor_copy(se[:], pvs[sub][D : D + 1, :])
                    bc = misc_p.tile([128, 512], F32, tag="miscp", name="bc")
                    nc.tensor.matmul(
                        bc[0:D, :], ones_row[:], se[:], start=True, stop=True
                    )
                    rb = rb_p.tile([D, 512], F32R, tag="rb", name="rb")
                    with nc.allow_low_precision(reason="f32r is full width"):
                        nc.vector.reciprocal(rb[:], bc[0:D, :])
                    nc.vector.tensor_tensor(
                        att[sub * 64 : sub * 64 + 64, hp, qs],
                        pvs[sub][0:D, :], rb[:], op=AluOpType.mult,
                    )
            deferred_norm = norm_pair
        if deferred_norm is not None:
            deferred_norm()
            deferred_norm = None
        # projection for this q-block
        for sc in range(4):
            sg = qb * 4 + sc
            ys = ys_p.tile([128, E], F32, tag="ys")
            for n0, nw in ((0, 512), (512, 256)):
                if qb == NQB - 1:
                    # attention done; reuse idle score-pool banks so the
                    # final projection isn't serialized on one slot
                    ya = st_p.tile([128, 512], F32, tag="st", name="ya")
                else:
                    ya = misc_p.tile([128, 512], F32, tag="miscp")
                for t in range(VC // 128):
                    nc.tensor.matmul(
                        ya[:, :nw],
                        att[:, t, sg * 128 : (sg + 1) * 128],
                        wp[:, t, n0 : n0 + nw],
                        start=(t == 0), stop=(t == VC // 128 - 1),
                    )
                nc.vector.tensor_copy(ys[:, n0 : n0 + nw], ya[:, :nw])
            nc.sync.dma_start(y_d.ap()[sg * 128 : (sg + 1) * 128, :], ys[:])

    for p in reversed(ctx_pools):
        p.__exit__(None, None, None)


def _emit_qkv(nc, x_d, ident, wqk, wv, bqk, qkT, vst, xs_p, xt_p, tp_p, va_p, qk_p):
    for sb in range(NSB):
        xt = xt_p.tile([128, KCH, 512], F32R)
        for sc in range(4):
            sg = sb * 4 + sc
            xs = xs_p.tile([128, E], F32)
            nc.sync.dma_start(xs[:], x_d.ap()[sg * 128 : (sg + 1) * 128, :])
            # batch 4 transposes per PSUM bank, then 3 -> one DVE copy each
            for g in range(2):
                kn = 4 if g == 0 else 2
                tp = tp_p.tile([128, 512], F32, tag="tp")
                for kk in range(kn):
                    k = g * 4 + kk
                    # 4 transposes share one PSUM bank as one accumulation
                    # group (disjoint columns, per-element has_written).
                    nc.tensor.matmul(
                        tp[:, kk * 128 : (kk + 1) * 128],
                        xs[:, k * 128 : (k + 1) * 128], ident[:],
                        is_transpose=True,
                        start=(kk == 0), stop=(kk == kn - 1),
                    )
                nc.vector.tensor_copy(
                    xt[:, g * 4 : g * 4 + kn, sc * 128 : (sc + 1) * 128],
                    tp[:, : kn * 128].rearrange("p (k f) -> p k f", k=kn),
                )
            va = va_p.tile([128, VC], F32)
            for k in range(KCH):
                nc.tensor.matmul(
                    va[:], xt[:, k, sc * 128 : (sc + 1) * 128], wv[:, k, :],
                    start=(k == 0), stop=(k == KCH - 1),
                )
            nc.vector.tensor_copy(
                vst[sb][:, sc, :, 0:D],
                va[:].rearrange("p (h d) -> p h d", h=HC),
            )
        for m in range(QKC // 128):
            qk = qk_p.tile([128, 512], F32)
            for k in range(KCH):
                nc.tensor.matmul(
                    qk[:], wqk[:, k, m * 128 : (m + 1) * 128], xt[:, k, :],
                    start=(k == 0), stop=(k == KCH - 1),
                )
            nc.vector.tensor_scalar_add(
                qkT[sb][:, m, :], qk[:], bqk[:, m : m + 1]
            )


def make_core_inputs(x, mask, Wqkv, bqkv):
    """Slice full inputs into 8 per-core input maps."""
    x = np.ascontiguousarray(np.asarray(x, dtype=np.float32))
    mask = np.asarray(mask)
    Wqkv = np.asarray(Wqkv, dtype=np.float32)
    bqkv = np.asarray(bqkv, dtype=np.float32)
    in_maps = []
    for c in range(8):
        b = c // 2
        h0 = (c % 2) * HC
        wq = Wqkv[:, h0 * D : (h0 + HC) * D]
        wk = Wqkv[:, E + h0 * D : E + (h0 + HC) * D]
        wqk = np.concatenate([wq, wk], axis=1).reshape(KCH, 128, QKC)
        wv = Wqkv[:, 2 * E + h0 * D : 2 * E + (h0 + HC) * D].reshape(KCH, 128, VC)
        bqk = np.concatenate(
            [bqkv[h0 * D : (h0 + HC) * D], bqkv[E + h0 * D : E + (h0 + HC) * D]]
        )
        mb = np.where(mask[b, 0, 0, :] == 0, np.float32(MASK_NEG), np.float32(0.0))
        in_maps.append(
            {
                "x": np.ascontiguousarray(x[b]),
                "wqk": np.ascontiguousarray(wqk),
                "wv": np.ascontiguousarray(wv),
                "wp": None,  # filled below (needs Wproj)
                "bqk": np.ascontiguousarray(bqk.astype(np.float32)),
                "mb": np.ascontiguousarray(mb.astype(np.float32)),
            }
        )
    return in_maps


def run(x, mask, Wqkv, bqkv, Wproj, bproj, trace=False, trace_cores=None):
    Wproj = np.asarray(Wproj, dtype=np.float32)
    bproj = np.asarray(bproj, dtype=np.float32)
    bqkv_np = np.asarray(bqkv, dtype=np.float32)
    in_maps = make_core_inputs(x, mask, Wqkv, bqkv_np)
    for c in range(8):
        h0 = (c % 2) * HC
        wp = Wproj[h0 * D : (h0 + HC) * D, :].reshape(VC // 128, 128, E)
        in_maps[c]["wp"] = np.ascontiguousarray(wp)

    nc = build_program()
    try:
        res = run_bass_kernel_spmd(
            nc, in_maps, core_ids=list(range(8)), trace=trace,
            trace_cores=trace_cores,
        )
    except Exception:
        # transient device wedge (e.g. NRT_EXEC_UNIT_UNRECOVERABLE) —
        # one retry is usually enough
        res = run_bass_kernel_spmd(
            nc, in_maps, core_ids=list(range(8)), trace=trace,
            trace_cores=trace_cores,
        )
    parts = [res.results[c]["y"] for c in range(8)]

    # host-folded bias: v-bias passes through softmax (weights sum to 1),
    # so y += bv @ Wproj + bproj, applied once per batch row.
    bv = bqkv_np[2 * E : 3 * E]
    bias_row = bv @ Wproj + bproj
    y = np.stack(
        [parts[2 * b] + parts[2 * b + 1] + bias_row for b in range(B)]
    ).astype(np.float32)
    return y, res


def kernel(x, mask, Wqkv, bqkv, Wproj, bproj):
    y, _ = run(x, mask, Wqkv, bqkv, Wproj, bproj, trace=False)
    return y



# revision 4
# speedup vs baseline: 1.3409x; 1.3409x over previous
"""Trainium2 Bass kernel for CodeAttention (B=4, S=2048, E=768, H=12).

Sharding: 8 cores = 4 batches x 2 head-groups (6 heads each).
Each core computes a partial projection output for its batch; the host
sums the two partials per batch and adds the (host-folded) bias row.

Key optimizations vs the naive formulation:
- Masked-key compaction: the padding mask drops ~half the keys, so the
  host gathers kept rows of x and the kernel only computes K/V, scores,
  exp and AV over PK (= kept rounded up to 128) key positions instead
  of S. Padded key columns get a -50 additive bias so exp() ~ 0.
- Host-side transposition: x arrives as x^T chunks, so no on-chip
  PE transposes are needed anywhere.
- The v-bias is host-folded (softmax weights sum to 1), and softmax
  denominators come from a ones-column in the V store, broadcast with
  a single K=2 matmul per head-pair.
"""

import sys

if "/opt/trn_rl_repo" not in sys.path:
    sys.path.insert(0, "/opt/trn_rl_repo")

import numpy as np

import concourse.bass as bass  # noqa: F401
import concourse.mybir as mybir
import concourse.tile as tile
from concourse import bacc
from concourse.alu_op_type import AluOpType
from concourse.bass_utils import run_bass_kernel_spmd

F32 = mybir.dt.float32
F32R = mybir.dt.float32r
Act = mybir.ActivationFunctionType

B, S, E, H, D = 4, 2048, 768, 12, 64
HC = 6                    # heads per core
KCH = E // 128            # contraction chunks over E = 6
NQB = S // 512            # q blocks of 512 = 4
MASK_NEG = -50.0
VW = D + 1                # v width incl. ones column = 65


def _kb_blocks(pk):
    """Split PK into moving-dim blocks >=256 wide (f32r full rate)."""
    blocks, off = [], 0
    rem = pk
    while rem > 0:
        if rem <= 512:
            w = rem
        elif rem - 384 >= 256:
            w = 384
        else:
            w = 512
        blocks.append((off, w))
        off += w
        rem -= w
    return blocks


def build_program(pk=1152):
    nc = bacc.Bacc("TRN2", target_bir_lowering=False, debug=False, num_devices=8)

    pkc = pk // 128
    xq_d = nc.dram_tensor("xq", [KCH, 128, S], F32R, kind="ExternalInput")
    xk_d = nc.dram_tensor("xk", [KCH, 128, pk], F32R, kind="ExternalInput")
    wq_d = nc.dram_tensor("wq", [KCH, 128, HC * D], F32R, kind="ExternalInput")
    wk_d = nc.dram_tensor("wk", [KCH, 128, HC * D], F32R, kind="ExternalInput")
    wv_d = nc.dram_tensor("wv", [KCH, 128, HC * D], F32R, kind="ExternalInput")
    wp_d = nc.dram_tensor("wp", [HC * D // 128, 128, E], F32R, kind="ExternalInput")
    bq_d = nc.dram_tensor("bq", [HC * D], F32, kind="ExternalInput")
    bk_d = nc.dram_tensor("bk", [HC * D], F32, kind="ExternalInput")
    mb_d = nc.dram_tensor("mb", [pk], F32, kind="ExternalInput")
    y_d = nc.dram_tensor("y", [S, E], F32, kind="ExternalOutput")

    with tile.TileContext(nc) as tc:
        _emit(nc, tc, pk, pkc, xq_d, xk_d, wq_d, wk_d, wv_d, wp_d,
              bq_d, bk_d, mb_d, y_d)
    nc.compile()
    return nc


def _emit(nc, tc, pk, pkc, xq_d, xk_d, wq_d, wk_d, wv_d, wp_d,
          bq_d, bk_d, mb_d, y_d):
    ctx_pools = []

    def pool(name, bufs, space="SBUF"):
        p = tc.tile_pool(name=name, bufs=bufs, space=space)
        ctx_pools.append(p)
        return p.__enter__()

    consts = pool("consts", 1)
    store = pool("store", 1)

    # weights on the SWDGE (gpsimd) queue so x loads on the sync HWDGE
    # queue aren't serialized behind them.
    wq = consts.tile([128, KCH, HC * D], F32R)
    wk = consts.tile([128, KCH, HC * D], F32R)
    wv = consts.tile([128, KCH, HC * D], F32R)
    wp = consts.tile([128, HC * D // 128, E], F32R)
    for k in range(KCH):
        nc.gpsimd.dma_start(wk[:, k, :], wk_d.ap()[k])
    for k in range(KCH):
        nc.gpsimd.dma_start(wv[:, k, :], wv_d.ap()[k])
    for k in range(KCH):
        nc.gpsimd.dma_start(wq[:, k, :], wq_d.ap()[k])
    for t in range(HC * D // 128):
        nc.gpsimd.dma_start(wp[:, t, :], wp_d.ap()[t])

    bq = consts.tile([128, HC * D // 128], F32)
    nc.scalar.dma_start(bq[:], bq_d.ap().rearrange("(c p) -> p c", p=128))
    bk = consts.tile([128, HC * D // 128], F32)
    nc.scalar.dma_start(bk[:], bk_d.ap().rearrange("(c p) -> p c", p=128))
    mb = consts.tile([128, pkc], F32)
    nc.scalar.dma_start(mb[:], mb_d.ap().rearrange("(c p) -> p c", p=128))

    ones_row_f = consts.tile([1, D], F32)
    nc.vector.memset(ones_row_f[:], 1.0)
    ones_row = consts.tile([1, D], F32R)
    nc.vector.tensor_copy(ones_row[:], ones_row_f[:])

    # persistent stores
    kT = store.tile([128, 3, pk], F32R)          # k^T, bias folded in
    qTs = store.tile([128, 3, S], F32R)          # q^T, bias folded in
    vst = store.tile([128, pkc, HC, VW], F32R)   # v per k-chunk + ones col
    att = store.tile([128, 3, S], F32R)          # normalized attn out ^T

    ones_f = consts.tile([128, pkc * HC], F32)
    nc.vector.memset(ones_f[:], 1.0)
    nc.vector.tensor_copy(
        vst[:, :, :, D : D + 1],
        ones_f[:].rearrange("p (a b one) -> p a b one", a=pkc, b=HC, one=1),
    )

    # ---- Phase 1a: K/V projections over kept keys ----
    kbs = _kb_blocks(pk)
    with (
        tc.tile_pool(name="xk", bufs=1) as xk_p,
        tc.tile_pool(name="kvp", bufs=3, space="PSUM") as kvp_p,
    ):
        xk = xk_p.tile([128, KCH, pk], F32R)
        for k in range(KCH):
            nc.sync.dma_start(xk[:, k, :], xk_d.ap()[k])
        for (off, w) in kbs:
            for m in range(3):
                kp = kvp_p.tile([128, 512], F32, tag="kvp")
                for k in range(KCH):
                    nc.tensor.matmul(
                        kp[:, :w], wk[:, k, m * 128 : (m + 1) * 128],
                        xk[:, k, off : off + w],
                        start=(k == 0), stop=(k == KCH - 1),
                    )
                nc.vector.tensor_scalar_add(
                    kT[:, m, off : off + w], kp[:, :w], bk[:, m : m + 1]
                )
            for kc in range(off // 128, (off + w) // 128):
                va = kvp_p.tile([128, 512], F32, tag="kvp")
                for k in range(KCH):
                    nc.tensor.matmul(
                        va[:, : HC * D],
                        xk[:, k, kc * 128 : (kc + 1) * 128], wv[:, k, :],
                        start=(k == 0), stop=(k == KCH - 1),
                    )
                nc.vector.tensor_copy(
                    vst[:, kc, :, 0:D],
                    va[:, : HC * D].rearrange("p (h d) -> p h d", h=HC),
                )

    # ---- Phase 1b: Q projection for all queries ----
    with (
        tc.tile_pool(name="xq", bufs=2) as xq_p,
        tc.tile_pool(name="qp", bufs=3, space="PSUM") as qp_p,
    ):
        for qb in range(NQB):
            xq = xq_p.tile([128, KCH, 512], F32R)
            for k in range(KCH):
                nc.sync.dma_start(
                    xq[:, k, :], xq_d.ap()[k][:, qb * 512 : (qb + 1) * 512]
                )
            for m in range(3):
                qp = qp_p.tile([128, 512], F32, tag="qp")
                for k in range(KCH):
                    nc.tensor.matmul(
                        qp[:], wq[:, k, m * 128 : (m + 1) * 128], xq[:, k, :],
                        start=(k == 0), stop=(k == KCH - 1),
                    )
                nc.vector.tensor_scalar_add(
                    qTs[:, m, qb * 512 : (qb + 1) * 512], qp[:], bq[:, m : m + 1]
                )

    # ---- Phase 2: attention + projection ----
    st_p = pool("st", 2, space="PSUM")       # [128,1024] = 2 banks each
    pv_p = pool("pv", 3, space="PSUM")
    misc_p = pool("miscp", 1, space="PSUM")  # bc slot
    pt_p = pool("pt", 3)
    se_p = pool("se", 2)
    rb_p = pool("rb", 2)
    ys_p = pool("ys", 2)

    for qb in range(NQB):
        qs = slice(qb * 512, (qb + 1) * 512)
        deferred_norm = None
        for hp in range(3):
            pvs = [
                pv_p.tile([128, 512], F32, tag="pv", name=f"pv{qb}_{hp}_{i}")
                for i in range(2)
            ]
            for kc in range(pkc):
                st = st_p.tile([128, 1024], F32, tag="st")
                for sub in range(2):
                    r0 = sub * D
                    nc.tensor.matmul(
                        st[:, sub * 512 : (sub + 1) * 512],
                        kT[r0 : r0 + D, hp, kc * 128 : (kc + 1) * 128],
                        qTs[r0 : r0 + D, hp, qs],
                        start=True, stop=True,
                    )
                pt = pt_p.tile([128, 1024], F32R, tag="pt")
                nc.scalar.activation(
                    pt[:], st[:], Act.Exp, bias=mb[:, kc : kc + 1], scale=0.125
                )
                for sub in range(2):
                    h = hp * 2 + sub
                    nc.tensor.matmul(
                        pvs[sub][0:VW, :], vst[:, kc, h, :],
                        pt[:, sub * 512 : (sub + 1) * 512],
                        start=(kc == 0), stop=(kc == pkc - 1),
                    )
                if kc == 1 and deferred_norm is not None:
                    deferred_norm()
                    deferred_norm = None

            def norm_pair(pvs=pvs, hp=hp, qs=qs):
                for sub in range(2):
                    se = se_p.tile([1, 512], F32R, tag="se", name="se")
                    nc.vector.tensor_copy(se[:], pvs[sub][D : D + 1, :])
                    bc = misc_p.tile([128, 512], F32, tag="miscp", name="bc")
                    nc.tensor.matmul(
                        bc[0:D, :], ones_row[:], se[:], start=True, stop=True
                    )
                    rb = rb_p.tile([D, 512], F32R, tag="rb", name="rb")
                    with nc.allow_low_precision(reason="f32r is full width"):
                        nc.vector.reciprocal(rb[:], bc[0:D, :])
                    nc.vector.tensor_tensor(
                        att[sub * D : (sub + 1) * D, hp, qs],
                        pvs[sub][0:D, :], rb[:], op=AluOpType.mult,
                    )

            deferred_norm = norm_pair
        if deferred_norm is not None:
            deferred_norm()
            deferred_norm = None
        # projection for this q-block (ya slots borrowed from the st pool)
        for sc in range(4):
            sg = qb * 4 + sc
            ys = ys_p.tile([128, E], F32, tag="ys")
            for n0, nw in ((0, 512), (512, 256)):
                ya = st_p.tile([128, 512], F32, tag="st", name="ya")
                for t in range(3):
                    nc.tensor.matmul(
                        ya[:, :nw],
                        att[:, t, sg * 128 : (sg + 1) * 128],
                        wp[:, t, n0 : n0 + nw],
                        start=(t == 0), stop=(t == 2),
                    )
                nc.vector.tensor_copy(ys[:, n0 : n0 + nw], ya[:, :nw])
            nc.sync.dma_start(y_d.ap()[sg * 128 : (sg + 1) * 128, :], ys[:])

    for p in reversed(ctx_pools):
        p.__exit__(None, None, None)


def make_core_inputs(x, mask, Wqkv, bqkv):
    """Slice full inputs into 8 per-core input maps; returns (maps, pk)."""
    x = np.ascontiguousarray(np.asarray(x, dtype=np.float32))
    mask = np.asarray(mask)
    Wqkv = np.asarray(Wqkv, dtype=np.float32)
    bqkv = np.asarray(bqkv, dtype=np.float32)

    keep = [np.nonzero(mask[b, 0, 0, :] != 0)[0] for b in range(B)]
    maxc = max(1, max(len(k) for k in keep))
    pk = min(((maxc + 127) // 128) * 128, S)

    in_maps = []
    for c in range(8):
        b = c // 2
        h0 = (c % 2) * HC
        idx = keep[b]
        xkp = np.zeros((pk, E), dtype=np.float32)
        xkp[: len(idx)] = x[b][idx]
        mb = np.full(pk, MASK_NEG, dtype=np.float32)
        mb[: len(idx)] = 0.0

        wq = Wqkv[:, h0 * D : (h0 + HC) * D]
        wk = Wqkv[:, E + h0 * D : E + (h0 + HC) * D]
        wv = Wqkv[:, 2 * E + h0 * D : 2 * E + (h0 + HC) * D]
        in_maps.append(
            {
                "xq": np.ascontiguousarray(x[b].T.reshape(KCH, 128, S)),
                "xk": np.ascontiguousarray(xkp.T.reshape(KCH, 128, pk)),
                "wq": np.ascontiguousarray(wq.reshape(KCH, 128, HC * D)),
                "wk": np.ascontiguousarray(wk.reshape(KCH, 128, HC * D)),
                "wv": np.ascontiguousarray(wv.reshape(KCH, 128, HC * D)),
                "wp": None,  # filled in run() (needs Wproj)
                "bq": np.ascontiguousarray(bqkv[h0 * D : (h0 + HC) * D]),
                "bk": np.ascontiguousarray(
                    bqkv[E + h0 * D : E + (h0 + HC) * D]
                ),
                "mb": mb,
            }
        )
    return in_maps, pk


def run(x, mask, Wqkv, bqkv, Wproj, bproj, trace=False, trace_cores=None):
    Wproj = np.asarray(Wproj, dtype=np.float32)
    bproj = np.asarray(bproj, dtype=np.float32)
    bqkv_np = np.asarray(bqkv, dtype=np.float32)
    in_maps, pk = make_core_inputs(x, mask, Wqkv, bqkv_np)
    for c in range(8):
        h0 = (c % 2) * HC
        wp = Wproj[h0 * D : (h0 + HC) * D, :].reshape(HC * D // 128, 128, E)
        in_maps[c]["wp"] = np.ascontiguousarray(wp)

    global LAST_PK
    LAST_PK = pk
    nc = build_program(pk)
    try:
        res = run_bass_kernel_spmd(
            nc, in_maps, core_ids=list(range(8)), trace=trace,
            trace_cores=trace_cores,
        )
    except Exception:
        # transient device wedge — one retry is usually enough
        res = run_bass_kernel_spmd(
            nc, in_maps, core_ids=list(range(8)), trace=trace,
            trace_cores=trace_cores,
        )
    parts = [res.results[c]["y"] for c in range(8)]

    # host-folded bias: v-bias passes through softmax (weights sum to 1),
    # so y += bv @ Wproj + bproj, applied once per batch row.
    bv = bqkv_np[2 * E : 3 * E]
    bias_row = bv @ Wproj + bproj
    y = np.stack(
        [parts[2 * b] + parts[2 * b + 1] + bias_row for b in range(B)]
    ).astype(np.float32)
    return y, res


LAST_PK = 1152


def kernel(x, mask, Wqkv, bqkv, Wproj, bproj):
    y, _ = run(x, mask, Wqkv, bqkv, Wproj, bproj, trace=False)
    return y
